# revision 11
# baseline (speedup 1.0000x reference)
"""CrossProductLayer kernel for Trainium2 (Bass/Tile), 8-core data parallel.

out[b, :] = concat(x[b]**2, x[b], 0.5 * x[b,i]*x[b,j] for i<j) * w

Full inputs:  x [16384, 128] f32, w [8384] f32.
Full output:  [16384, 8384] f32.

Sharding: pure data parallel on the batch dim — each of the 8 cores gets
2048 rows of x; w (pre-scaled by the 0.5 pair factor and pre-broadcast to
[128, 8384] on host) is replicated. Forward only, no collectives.

Per-core device kernel. The core's 2048 rows = 16 row-tiles of 128
(partition dim). Work is organized in 8 units = (4 tile-groups of G=4
row-tiles) x (2 column halves of 4192 cols). A unit's SBUF tile is
[128, 4 x 4192] (67 KB/partition, bufs=2): storing it writes 16.8 KB
contiguous HBM rows, which sustains ~428 GB/s (narrow column chunks
measured as low as 198 GB/s — row length is the DMA efficiency lever).

Within a unit, each pair-block piece (out[:, blk] = x[:,i] * x[:,i+1:])
is one grouped op over [128, 4, w] APs. Engine split (calibrated costs):
  - ScalarE:  the widest blocks of EACH half (per-tile activation with
              per-partition scale), plus the squares block.
  - GpSimdE:  middle blocks (grouped tensor_tensor, broadcast operand in
              src1 — step-0-innermost in src0 is a slow path) + the *w
              pass over its own columns.
  - VectorE:  narrow tail + *w over its own columns + *w over ScalarE's
              columns (issued one unit late to avoid head-of-line
              blocking on the VectorE queue).
Each engine multiplies its own columns by w on its own queue, so no
cross-engine dependency exists until the store. VectorE issues only
tensor_tensor ops (1-port) so GpSimdE never contends for the shared
SBUF port. Singles arrive by DMA; the store is one HWDGE DMA per unit.
"""

import numpy as np

B = 16384
NI = 128
NF = NI + NI + (NI * (NI - 1)) // 2  # 8384
NCORES = 8
ROWS = B // NCORES  # 2048
TILE_P = 128
TILES = ROWS // TILE_P  # 16
PAIRS_OFF = 2 * NI  # 256

G = 4  # row-tiles per group
NG = TILES // G  # 4 groups
HALF = NF // 2  # 4192

WIDTHS = [NI - 1 - i for i in range(NI - 1)]
STARTS = []
_off = PAIRS_OFF
for _w in WIDTHS:
    STARTS.append(_off)
    _off += _w
assert _off == NF

# first block whose columns lie (partly) in half 1
I_H1 = next(i for i in range(NI - 1) if STARTS[i] + WIDTHS[i] > HALF)


def _pieces(h):
    lo, hi = h * HALF, (h + 1) * HALF
    out = []
    for i in range(NI - 1):
        s, w = STARTS[i], WIDTHS[i]
        a, b = max(s, lo), min(s + w, hi)
        if a < b:
            out.append((i, a, b - a))
    return out


# per-piece cost model (ns), totals over all 16 tiles / 4 units
ACT_PIECE = lambda w: 16 * (445.0 + w) / 1.2
GP_PIECE = lambda w: 4 * 300.0 + 32.0 * w
DVE_PIECE = lambda w: 4 * 290.0 + 16.7 * w
GP_WCOL = 32.0
DVE_WCOL = 16.7


def _plan():
    """Assign each piece to an engine: per half, [ACT widest | GP | DVE]
    in block order. Scan A0, A1, B0; solve B1 by GP/DVE balance."""
    p0, p1 = _pieces(0), _pieces(1)
    h1_act_start = next(k for k, (i, _, _) in enumerate(p1) if i >= I_H1)
    best = None
    for A0 in range(8, 34):
        for A1 in range(0, 40):
            act = 4 * (445 + 512) / 1.2  # grouped squares
            act += sum(ACT_PIECE(w) for _, _, w in p0[:A0])
            act += sum(
                ACT_PIECE(w)
                for _, _, w in p1[h1_act_start : h1_act_start + A1]
            )
            a_cols = sum(w for _, _, w in p0[:A0]) + sum(
                w for _, _, w in p1[h1_act_start : h1_act_start + A1]
            )
            rest = []
            rest += p0[A0:]
            rest += p1[:h1_act_start]
            rest += p1[h1_act_start + A1 :]
            # GP takes the widest of the rest, DVE the narrowest
            rest_sorted = sorted(rest, key=lambda t: -t[2])
            # scan GP count
            gp_load = 0.0
            gp_cols = 0
            loads = []
            for k in range(len(rest_sorted) + 1):
                d_cols = sum(w for _, _, w in rest_sorted[k:])
                dve = sum(DVE_PIECE(w) for _, _, w in rest_sorted[k:])
                dve += DVE_WCOL * (d_cols + a_cols + PAIRS_OFF)
                gp = gp_load + GP_WCOL * gp_cols
                loads.append((max(act, gp, dve), k, gp, dve))
                if k < len(rest_sorted):
                    gp_load += GP_PIECE(rest_sorted[k][2])
                    gp_cols += rest_sorted[k][2]
            mk, k, gp, dve = min(loads)
            if best is None or mk < best[0]:
                gp_set = {(i, cs) for i, cs, _ in rest_sorted[:k]}
                best = (mk, A0, A1, gp_set, act, gp, dve)
    return best


_MK, _A0, _A1, _GPSET, _ACT_L, _GP_L, _DVE_L = _plan()


def _assign(h):
    """Ordered (piece, engine) for half h."""
    ps = _pieces(h)
    if h == 0:
        act_idx = set(range(_A0))
        act_blocks = {ps[k][0] for k in act_idx}
    else:
        h1s = next(k for k, (i, _, _) in enumerate(ps) if i >= I_H1)
        act_blocks = {ps[k][0] for k in range(h1s, h1s + _A1)}
    out = []
    for i, cs, w in ps:
        if i in act_blocks:
            e = "A"
        elif (i, cs) in _GPSET:
            e = "G"
        else:
            e = "D"
        out.append((i, cs, w, e))
    return out


ASSIGN = [_assign(0), _assign(1)]


def _w_runs(h):
    """Contiguous same-engine column runs for the *w pass, half h."""
    runs = []
    if h == 0:
        runs.append(["A", 0, PAIRS_OFF])  # squares+singles head -> DVE(Wa)
    for i, cs, w, e in ASSIGN[h]:
        if runs and runs[-1][0] == e and runs[-1][2] == cs:
            runs[-1][2] = cs + w
        else:
            runs.append([e, cs, cs + w])
    return [(e, a, b) for e, a, b in runs]


W_RUNS = [_w_runs(0), _w_runs(1)]

_CACHE = {}


def _build_nc():
    from concourse import bacc
    import concourse.mybir as mybir
    from concourse.tile import TileContext

    f32 = mybir.dt.float32
    nc = bacc.Bacc(
        "TRN2",
        target_bir_lowering=False,
        debug=False,
        num_devices=NCORES,
    )
    x_d = nc.dram_tensor("x", [ROWS, NI], f32, kind="ExternalInput")
    w_d = nc.dram_tensor("w", [NI, NF], f32, kind="ExternalInput")
    o_d = nc.dram_tensor("out", [ROWS, NF], f32, kind="ExternalOutput")

    x_hbm3 = x_d.rearrange("(t p) c -> p t c", t=TILES)
    o_hbm3 = o_d.rearrange("(t p) c -> p t c", t=TILES)

    units = [(g, h) for g in range(NG) for h in range(2)]

    with TileContext(nc) as tc:
        with (
            tc.tile_pool(name="xp", bufs=1) as xp,
            tc.tile_pool(name="wp", bufs=1) as wp,
            tc.tile_pool(name="pp", bufs=2) as pp,
        ):
            x_all = xp.tile([TILE_P, TILES * NI], f32)
            x3 = x_all[:].rearrange("p (t c) -> p t c", t=TILES)
            nc.sync.dma_start(out=x3, in_=x_hbm3)
            w_t = wp.tile([NI, NF], f32)
            nc.sync.dma_start(out=w_t[:], in_=w_d[:])

            pans = {}

            def finish(k):
                """Wa pass + store for unit k (issued one unit late)."""
                g, h = units[k]
                t0, t1 = g * G, (g + 1) * G
                lo = h * HALF
                pan3 = pans[k]
                for e, a, b in W_RUNS[h]:
                    if e != "A":
                        continue
                    wsl = w_t[:, None, a:b].broadcast_to(
                        [TILE_P, G, b - a]
                    )
                    nc.vector.tensor_mul(
                        pan3[:, :, a - lo : b - lo],
                        pan3[:, :, a - lo : b - lo],
                        wsl,
                    )
                nc.sync.dma_start(
                    out=o_hbm3[:, t0:t1, lo : lo + HALF], in_=pan3
                )
                del pans[k]

            for k, (g, h) in enumerate(units):
                t0, t1 = g * G, (g + 1) * G
                lo = h * HALF
                pan = pp.tile(
                    [TILE_P, G * HALF], f32, name=f"pan{g}{h}", tag="pan"
                )
                pan3 = pan[:].rearrange("p (t c) -> p t c", t=G)
                pans[k] = pan3
                if h == 0:
                    nc.sync.dma_start(
                        out=pan3[:, :, NI : 2 * NI], in_=x_hbm3[:, t0:t1]
                    )
                    nc.scalar.square(pan3[:, :, 0:NI], x3[:, t0:t1])
                for i, cs, w, e in ASSIGN[h]:
                    c = cs - lo
                    j0 = i + 1 + (cs - STARTS[i])
                    src = x3[:, t0:t1, j0 : j0 + w]
                    dst = pan3[:, :, c : c + w]
                    if e == "A":
                        for t in range(G):
                            nc.scalar.mul(
                                dst[:, t],
                                src[:, t],
                                x3[:, t0 + t, i : i + 1],
                            )
                    else:
                        bc = x3[:, t0:t1, i : i + 1].broadcast_to(
                            [TILE_P, G, w]
                        )
                        if e == "G":
                            nc.gpsimd.tensor_mul(dst, src, bc)
                        else:
                            nc.vector.tensor_mul(dst, bc, src)
                # own-column *w passes on the owning engine's queue
                for e, a, b in W_RUNS[h]:
                    if e == "A":
                        continue
                    wsl = w_t[:, None, a:b].broadcast_to(
                        [TILE_P, G, b - a]
                    )
                    eng = nc.gpsimd if e == "G" else nc.vector
                    eng.tensor_mul(
                        pan3[:, :, a - lo : b - lo],
                        pan3[:, :, a - lo : b - lo],
                        wsl,
                    )
                if k >= 1:
                    finish(k - 1)
            finish(len(units) - 1)
    nc.compile()
    return nc


def _get_nc():
    if "nc" not in _CACHE:
        _CACHE["nc"] = _build_nc()
    return _CACHE["nc"]


def _prep_in_maps(x, w):
    x = np.ascontiguousarray(np.asarray(x, dtype=np.float32))
    w = np.asarray(w, dtype=np.float32)
    w_scaled = w.copy()
    w_scaled[PAIRS_OFF:] *= np.float32(0.5)
    w_b = np.ascontiguousarray(np.broadcast_to(w_scaled[None, :], (NI, NF)))
    return [
        {"x": np.ascontiguousarray(x[c * ROWS : (c + 1) * ROWS]), "w": w_b}
        for c in range(NCORES)
    ]


def _run(x, w, trace=False, tmpdir=None):
    from concourse.bass_utils import run_bass_kernel_spmd

    nc = _get_nc()
    in_maps = _prep_in_maps(x, w)
    res = run_bass_kernel_spmd(
        nc, in_maps, list(range(NCORES)), trace=trace, tmpdir=tmpdir
    )
    out = np.concatenate([res.results[c]["out"] for c in range(NCORES)], axis=0)
    return out, res


def kernel(**inputs):
    out, _ = _run(inputs["x"], inputs["w"])
    return out


if __name__ == "__main__":
    print(f"A0={_A0} A1={_A1}")
    print(
        f"pred ACT={_ACT_L/1e3:.0f}us GP={_GP_L/1e3:.0f}us "
        f"DVE={_DVE_L/1e3:.0f}us makespan={_MK/1e3:.0f}us"
    )
    for h in range(2):
        eng_cols = {}
        for i, cs, w, e in ASSIGN[h]:
            eng_cols[e] = eng_cols.get(e, 0) + w
        print(f"half{h} cols by engine: {eng_cols}")
        print(f"half{h} w-runs: {W_RUNS[h]}")


# revision 14
# speedup vs baseline: 1.1071x; 1.1071x over previous
"""CrossProductLayer kernel for Trainium2 (Bass/Tile), 8-core data parallel.

out[b, :] = concat(x[b]**2, x[b], 0.5 * x[b,i]*x[b,j] for i<j) * w

Full inputs:  x [16384, 128] f32, w [8384] f32.
Full output:  [16384, 8384] f32.

Sharding: pure data parallel on the batch dim — each of the 8 cores gets
2048 rows of x; w (pre-scaled by the 0.5 pair factor and pre-broadcast to
[128, 8384] on host) is replicated. Forward only, no collectives.

Per-core device kernel. The core's 2048 rows = 16 row-tiles of 128
(partition dim). Work is organized in 8 units = (4 tile-groups of G=4
row-tiles) x (2 column halves of 4192 cols). A unit's SBUF tile is
[128, 4 x 4192] (67 KB/partition, bufs=2): storing it writes 16.8 KB
contiguous HBM rows, which sustains ~428 GB/s (narrow column chunks
measured as low as 198 GB/s — row length is the DMA efficiency lever).

Within a unit, each pair-block piece (out[:, blk] = x[:,i] * x[:,i+1:])
is one grouped op over [128, 4, w] APs. Engine split (calibrated costs):
  - ScalarE:  the widest blocks of EACH half (per-tile activation with
              per-partition scale), plus the squares block.
  - GpSimdE:  middle blocks (grouped tensor_tensor, broadcast operand in
              src1 — step-0-innermost in src0 is a slow path) + the *w
              pass over its own columns.
  - VectorE:  narrow tail + *w over its own columns + *w over ScalarE's
              columns (issued one unit late to avoid head-of-line
              blocking on the VectorE queue).
Each engine multiplies its own columns by w on its own queue, so no
cross-engine dependency exists until the store. VectorE issues only
tensor_tensor ops (1-port) so GpSimdE never contends for the shared
SBUF port. Singles arrive by DMA; the store is one HWDGE DMA per unit.
"""

import numpy as np

B = 16384
NI = 128
NF = NI + NI + (NI * (NI - 1)) // 2  # 8384
NCORES = 8
ROWS = B // NCORES  # 2048
TILE_P = 128
TILES = ROWS // TILE_P  # 16
PAIRS_OFF = 2 * NI  # 256

G = 4  # row-tiles per group
NG = TILES // G  # 4 groups
HALF = NF // 2  # 4192

WIDTHS = [NI - 1 - i for i in range(NI - 1)]
STARTS = []
_off = PAIRS_OFF
for _w in WIDTHS:
    STARTS.append(_off)
    _off += _w
assert _off == NF

# first block whose columns lie (partly) in half 1
I_H1 = next(i for i in range(NI - 1) if STARTS[i] + WIDTHS[i] > HALF)


def _pieces(h):
    lo, hi = h * HALF, (h + 1) * HALF
    out = []
    for i in range(NI - 1):
        s, w = STARTS[i], WIDTHS[i]
        a, b = max(s, lo), min(s + w, hi)
        if a < b:
            out.append((i, a, b - a))
    return out


# per-piece cost model (ns), totals over all 16 tiles / 4 units
ACT_PIECE = lambda w: 16 * (445.0 + w) / 1.2
GP_PIECE = lambda w: 4 * 300.0 + 32.0 * w
DVE_PIECE = lambda w: 4 * 290.0 + 16.7 * w
GP_WCOL = 32.0
DVE_WCOL = 16.7


def _plan():
    """Assign each piece to an engine: per half, [ACT widest | GP | DVE]
    in block order. Scan A0, A1, B0; solve B1 by GP/DVE balance."""
    p0, p1 = _pieces(0), _pieces(1)
    h1_act_start = next(k for k, (i, _, _) in enumerate(p1) if i >= I_H1)
    best = None
    for A0 in range(8, 34):
        for A1 in range(0, 40):
            act = 4 * (445 + 512) / 1.2  # grouped squares
            act += sum(ACT_PIECE(w) for _, _, w in p0[:A0])
            act += sum(
                ACT_PIECE(w)
                for _, _, w in p1[h1_act_start : h1_act_start + A1]
            )
            a_cols = sum(w for _, _, w in p0[:A0]) + sum(
                w for _, _, w in p1[h1_act_start : h1_act_start + A1]
            )
            rest = []
            rest += p0[A0:]
            rest += p1[:h1_act_start]
            rest += p1[h1_act_start + A1 :]
            # GP takes the widest of the rest, DVE the narrowest
            rest_sorted = sorted(rest, key=lambda t: -t[2])
            # scan GP count
            gp_load = 0.0
            gp_cols = 0
            loads = []
            for k in range(len(rest_sorted) + 1):
                d_cols = sum(w for _, _, w in rest_sorted[k:])
                dve = sum(DVE_PIECE(w) for _, _, w in rest_sorted[k:])
                dve += DVE_WCOL * (d_cols + a_cols + PAIRS_OFF)
                gp = gp_load + GP_WCOL * gp_cols
                loads.append((max(act, gp, dve), k, gp, dve))
                if k < len(rest_sorted):
                    gp_load += GP_PIECE(rest_sorted[k][2])
                    gp_cols += rest_sorted[k][2]
            mk, k, gp, dve = min(loads)
            if best is None or mk < best[0]:
                gp_set = {(i, cs) for i, cs, _ in rest_sorted[:k]}
                best = (mk, A0, A1, gp_set, act, gp, dve)
    return best


_MK, _A0, _A1, _GPSET, _ACT_L, _GP_L, _DVE_L = _plan()


def _assign(h):
    """Ordered (piece, engine) for half h."""
    ps = _pieces(h)
    if h == 0:
        act_idx = set(range(_A0))
        act_blocks = {ps[k][0] for k in act_idx}
    else:
        h1s = next(k for k, (i, _, _) in enumerate(ps) if i >= I_H1)
        act_blocks = {ps[k][0] for k in range(h1s, h1s + _A1)}
    out = []
    for i, cs, w in ps:
        if i in act_blocks:
            e = "A"
        elif (i, cs) in _GPSET:
            e = "G"
        else:
            e = "D"
        out.append((i, cs, w, e))
    return out


ASSIGN = [_assign(0), _assign(1)]


def _w_runs(h):
    """Contiguous same-engine column runs for the *w pass, half h."""
    runs = []
    if h == 0:
        runs.append(["A", 0, PAIRS_OFF])  # squares+singles head -> DVE(Wa)
    for i, cs, w, e in ASSIGN[h]:
        if runs and runs[-1][0] == e and runs[-1][2] == cs:
            runs[-1][2] = cs + w
        else:
            runs.append([e, cs, cs + w])
    return [(e, a, b) for e, a, b in runs]


W_RUNS = [_w_runs(0), _w_runs(1)]

_CACHE = {}


def _build_nc():
    import os

    os.environ["TILE_EXHAUSTIVE_MEMORY_SHARE_CHECK"] = "1"
    from concourse import bacc
    import concourse.mybir as mybir
    from concourse.tile import TileContext

    f32 = mybir.dt.float32
    nc = bacc.Bacc(
        "TRN2",
        target_bir_lowering=False,
        debug=False,
        num_devices=NCORES,
    )
    x_d = nc.dram_tensor("x", [ROWS, NI], f32, kind="ExternalInput")
    w_d = nc.dram_tensor("w", [NI, NF], f32, kind="ExternalInput")
    o_d = nc.dram_tensor("out", [ROWS, NF], f32, kind="ExternalOutput")

    x_hbm3 = x_d.rearrange("(t p) c -> p t c", t=TILES)
    o_hbm3 = o_d.rearrange("(t p) c -> p t c", t=TILES)

    units = [(g, h) for g in range(NG) for h in range(2)]

    with TileContext(nc) as tc:
        with (
            tc.tile_pool(name="xp", bufs=1) as xp,
            tc.tile_pool(name="wp", bufs=1) as wp,
            tc.tile_pool(name="pp", bufs=2) as pp,
        ):
            x_all = xp.tile([TILE_P, TILES * NI], f32)
            x3 = x_all[:].rearrange("p (t c) -> p t c", t=TILES)
            nc.sync.dma_start(out=x3, in_=x_hbm3)
            w_t = wp.tile([NI, NF], f32)
            nc.sync.dma_start(out=w_t[:], in_=w_d[:])

            pans = {}

            def finish(k):
                """Wa pass + store for unit k (issued one unit late)."""
                g, h = units[k]
                t0, t1 = g * G, (g + 1) * G
                lo = h * HALF
                pan3 = pans[k]
                for e, a, b in W_RUNS[h]:
                    if e != "A":
                        continue
                    wsl = w_t[:, None, a:b].broadcast_to(
                        [TILE_P, G, b - a]
                    )
                    nc.vector.tensor_mul(
                        pan3[:, :, a - lo : b - lo],
                        pan3[:, :, a - lo : b - lo],
                        wsl,
                    )
                nc.sync.dma_start(
                    out=o_hbm3[:, t0:t1, lo : lo + HALF], in_=pan3
                )
                del pans[k]

            for k, (g, h) in enumerate(units):
                if k >= 1:
                    finish(k - 1)
                t0, t1 = g * G, (g + 1) * G
                lo = h * HALF
                pan = pp.tile(
                    [TILE_P, G * HALF], f32, name=f"pan{g}{h}", tag="pan"
                )
                pan3 = pan[:].rearrange("p (t c) -> p t c", t=G)
                pans[k] = pan3
                if h == 0:
                    nc.sync.dma_start(
                        out=pan3[:, :, NI : 2 * NI], in_=x_hbm3[:, t0:t1]
                    )
                    nc.scalar.square(pan3[:, :, 0:NI], x3[:, t0:t1])
                for i, cs, w, e in ASSIGN[h]:
                    c = cs - lo
                    j0 = i + 1 + (cs - STARTS[i])
                    src = x3[:, t0:t1, j0 : j0 + w]
                    dst = pan3[:, :, c : c + w]
                    if e == "A":
                        for t in range(G):
                            nc.scalar.mul(
                                dst[:, t],
                                src[:, t],
                                x3[:, t0 + t, i : i + 1],
                            )
                    else:
                        bc = x3[:, t0:t1, i : i + 1].broadcast_to(
                            [TILE_P, G, w]
                        )
                        if e == "G":
                            nc.gpsimd.tensor_mul(dst, src, bc)
                        else:
                            nc.vector.tensor_mul(dst, bc, src)
                # own-column *w passes on the owning engine's queue
                for e, a, b in W_RUNS[h]:
                    if e == "A":
                        continue
                    wsl = w_t[:, None, a:b].broadcast_to(
                        [TILE_P, G, b - a]
                    )
                    eng = nc.gpsimd if e == "G" else nc.vector
                    eng.tensor_mul(
                        pan3[:, :, a - lo : b - lo],
                        pan3[:, :, a - lo : b - lo],
                        wsl,
                    )
            finish(len(units) - 1)
    nc.compile()
    return nc


def _get_nc():
    if "nc" not in _CACHE:
        _CACHE["nc"] = _build_nc()
    return _CACHE["nc"]


def _prep_in_maps(x, w):
    x = np.ascontiguousarray(np.asarray(x, dtype=np.float32))
    w = np.asarray(w, dtype=np.float32)
    w_scaled = w.copy()
    w_scaled[PAIRS_OFF:] *= np.float32(0.5)
    w_b = np.ascontiguousarray(np.broadcast_to(w_scaled[None, :], (NI, NF)))
    return [
        {"x": np.ascontiguousarray(x[c * ROWS : (c + 1) * ROWS]), "w": w_b}
        for c in range(NCORES)
    ]


def _run(x, w, trace=False, tmpdir=None):
    from concourse.bass_utils import run_bass_kernel_spmd

    nc = _get_nc()
    in_maps = _prep_in_maps(x, w)
    res = run_bass_kernel_spmd(
        nc, in_maps, list(range(NCORES)), trace=trace, tmpdir=tmpdir
    )
    out = np.concatenate([res.results[c]["out"] for c in range(NCORES)], axis=0)
    return out, res


def kernel(**inputs):
    out, _ = _run(inputs["x"], inputs["w"])
    return out


if __name__ == "__main__":
    print(f"A0={_A0} A1={_A1}")
    print(
        f"pred ACT={_ACT_L/1e3:.0f}us GP={_GP_L/1e3:.0f}us "
        f"DVE={_DVE_L/1e3:.0f}us makespan={_MK/1e3:.0f}us"
    )
    for h in range(2):
        eng_cols = {}
        for i, cs, w, e in ASSIGN[h]:
            eng_cols[e] = eng_cols.get(e, 0) + w
        print(f"half{h} cols by engine: {eng_cols}")
        print(f"half{h} w-runs: {W_RUNS[h]}")


# revision 15
# speedup vs baseline: 1.2296x; 1.1107x over previous
"""CrossProductLayer kernel for Trainium2 (Bass/Tile), 8-core data parallel.

out[b, :] = concat(x[b]**2, x[b], 0.5 * x[b,i]*x[b,j] for i<j) * w

Full inputs:  x [16384, 128] f32, w [8384] f32 -> output [16384, 8384] f32.
Pure batch data parallelism: each of 8 cores computes 2048 rows; w is
pre-scaled (0.5 on the pair block) and pre-broadcast to [128, 8384] host-side.

Per-core kernel: 16 row-tiles of 128 rows. Units = (2 groups of G=8
row-tiles) x (4 column chunks of 2096). Unit tile [128, 8*2096] f32
(67 KB/partition, bufs=2). Stores write 8.4 KB HBM rows (~306 GB/s).

Each pair block i (out[:, blk] = x[:,i]*x[:,i+1:]) is one grouped op per
unit over [128, 8, w]. Engine split via measured per-op costs:
  ScalarE: widest blocks of every chunk (per-tile ops), capped budget;
           squares; singles by DMA.
  GpSimdE: narrow blocks (low fixed cost/op) + share of the *w pass.
  VectorE: wide/middle blocks + rest of the *w pass + ScalarE-cols *w.
All VectorE ops are tensor_tensor (1-port) so GpSimdE never contends on
the shared SBUF port; the GpSimd broadcast operand is src1 (src0 step-0
innermost is a slow path). Per-unit *w passes run on the owning engine's
own queue; stores issue early (next loop iteration head).
"""

import numpy as np

B = 16384
NI = 128
NF = NI + NI + (NI * (NI - 1)) // 2  # 8384
NCORES = 8
ROWS = B // NCORES
TILE_P = 128
TILES = ROWS // TILE_P  # 16
PAIRS_OFF = 2 * NI

G = 8
NG = TILES // G  # 2 groups
NCH = 4
CHW = NF // NCH  # 2096

WIDTHS = [NI - 1 - i for i in range(NI - 1)]
STARTS = []
_off = PAIRS_OFF
for _w in WIDTHS:
    STARTS.append(_off)
    _off += _w
assert _off == NF

# fitted per-op costs (ns); per unit (G rows of one group)
ACT_OP = lambda w: 371.0 + 0.83 * w  # per tile => G per unit per block
DVE_OP = lambda w: 620.0 + 1.09 * G * w
GP_OP = lambda w: 390.0 + 1.93 * G * w
DVE_WCOL = 1.09 * G  # per col per unit
GP_WCOL = 1.93 * G

ACT_BUDGET = 160e3  # ns total across all units


def _pieces(ch):
    lo, hi = ch * CHW, (ch + 1) * CHW
    out = []
    for i in range(NI - 1):
        s, w = STARTS[i], WIDTHS[i]
        a, b = max(s, lo), min(s + w, hi)
        if a < b:
            out.append((i, a, b - a))
    return out


def _plan():
    """Per chunk: ACT takes widest (within global cap), GP narrow, DVE
    wide; per-chunk W split balances DVE vs GP."""
    plan = []
    act_per_chunk = ACT_BUDGET / NCH
    for ch in range(NCH):
        ps = _pieces(ch)
        by_w = sorted(ps, key=lambda t: -t[2])
        a_load = 16 * (371 + 128) / 1.2 / NCH if ch == 0 else 0.0
        act_set = set()
        for i, cs, w in by_w:
            c = 16 * ACT_OP(w)
            if a_load + c <= act_per_chunk:
                act_set.add((i, cs))
                a_load += c
        # remaining: GP narrow, DVE wide — threshold by per-op balance
        rest = [(i, cs, w) for i, cs, w in ps if (i, cs) not in act_set]
        d_load, g_load = 0.0, 0.0
        assign = {}
        for i, cs, w in sorted(rest, key=lambda t: -t[2]):
            cd, cg = NG * TILES / G * 0 + 2 * DVE_OP(w), 2 * GP_OP(w)
            if d_load + cd <= g_load + cg:
                assign[(i, cs)] = "D"
                d_load += cd
            else:
                assign[(i, cs)] = "G"
                g_load += cg
        # per-chunk W cols (incl head in chunk 0) split D/G
        wcols = CHW
        wd = (g_load - d_load + 2 * GP_WCOL * wcols) / (
            2 * (DVE_WCOL + GP_WCOL)
        )
        wd = int(np.clip(round(wd), 0, wcols))
        d_load += 2 * DVE_WCOL * wd
        g_load += 2 * GP_WCOL * (wcols - wd)
        plan.append(
            {
                "ch": ch,
                "pieces": ps,
                "act": act_set,
                "assign": assign,
                "wd": wd,
                "loads": (a_load, g_load, d_load),
            }
        )
    return plan


PLAN = _plan()

_CACHE = {}


def _build_nc():
    import os

    os.environ["TILE_EXHAUSTIVE_MEMORY_SHARE_CHECK"] = "1"
    from concourse import bacc
    import concourse.mybir as mybir
    from concourse.tile import TileContext

    f32 = mybir.dt.float32
    nc = bacc.Bacc(
        "TRN2", target_bir_lowering=False, debug=False, num_devices=NCORES
    )
    x_d = nc.dram_tensor("x", [ROWS, NI], f32, kind="ExternalInput")
    w_d = nc.dram_tensor("w", [NI, NF], f32, kind="ExternalInput")
    o_d = nc.dram_tensor("out", [ROWS, NF], f32, kind="ExternalOutput")

    x_hbm3 = x_d.rearrange("(t p) c -> p t c", t=TILES)
    o_hbm3 = o_d.rearrange("(t p) c -> p t c", t=TILES)

    units = [(g, ch) for g in range(NG) for ch in range(NCH)]

    with TileContext(nc) as tc:
        with (
            tc.tile_pool(name="xp", bufs=1) as xp,
            tc.tile_pool(name="wp", bufs=1) as wp,
            tc.tile_pool(name="pp", bufs=2) as pp,
        ):
            x_all = xp.tile([TILE_P, TILES * NI], f32)
            x3 = x_all[:].rearrange("p (t c) -> p t c", t=TILES)
            nc.sync.dma_start(out=x3, in_=x_hbm3)
            w_t = wp.tile([NI, NF], f32)
            nc.sync.dma_start(out=w_t[:], in_=w_d[:])

            pending = []  # (pan3, t0, t1, lo)

            def flush():
                while pending:
                    pan3, t0, t1, lo = pending.pop(0)
                    nc.sync.dma_start(
                        out=o_hbm3[:, t0:t1, lo : lo + CHW], in_=pan3
                    )

            for k, (g, ch) in enumerate(units):
                flush()
                info = PLAN[ch]
                t0, t1 = g * G, (g + 1) * G
                lo = ch * CHW
                pan = pp.tile(
                    [TILE_P, G * CHW], f32, name=f"pan{g}{ch}", tag="pan"
                )
                pan3 = pan[:].rearrange("p (t c) -> p t c", t=G)
                if ch == 0:
                    nc.sync.dma_start(
                        out=pan3[:, :, NI : 2 * NI], in_=x_hbm3[:, t0:t1]
                    )
                    nc.scalar.square(pan3[:, :, 0:NI], x3[:, t0:t1])
                # issue ACT first (longest serial stream), then GP, DVE
                for eng_sel in ("A", "G", "D"):
                    for i, cs, w in info["pieces"]:
                        key = (i, cs)
                        e = (
                            "A"
                            if key in info["act"]
                            else info["assign"][key]
                        )
                        if e != eng_sel:
                            continue
                        c = cs - lo
                        j0 = i + 1 + (cs - STARTS[i])
                        src = x3[:, t0:t1, j0 : j0 + w]
                        dst = pan3[:, :, c : c + w]
                        if e == "A":
                            for t in range(G):
                                nc.scalar.mul(
                                    dst[:, t],
                                    src[:, t],
                                    x3[:, t0 + t, i : i + 1],
                                )
                        else:
                            bc = x3[:, t0:t1, i : i + 1].broadcast_to(
                                [TILE_P, G, w]
                            )
                            if e == "G":
                                nc.gpsimd.tensor_mul(dst, src, bc)
                            else:
                                nc.vector.tensor_mul(dst, bc, src)
                # *w pass split D/G by the planner
                wd = info["wd"]
                wsl = w_t[:, None, lo : lo + CHW]
                if wd > 0:
                    nc.vector.tensor_mul(
                        pan3[:, :, 0:wd],
                        pan3[:, :, 0:wd],
                        wsl[:, :, 0:wd].broadcast_to([TILE_P, G, wd]),
                    )
                if wd < CHW:
                    nc.gpsimd.tensor_mul(
                        pan3[:, :, wd:CHW],
                        pan3[:, :, wd:CHW],
                        wsl[:, :, wd:CHW].broadcast_to(
                            [TILE_P, G, CHW - wd]
                        ),
                    )
                pending.append((pan3, t0, t1, lo))
            flush()
    nc.compile()
    return nc


def _get_nc():
    if "nc" not in _CACHE:
        _CACHE["nc"] = _build_nc()
    return _CACHE["nc"]


def _prep_in_maps(x, w):
    x = np.ascontiguousarray(np.asarray(x, dtype=np.float32))
    w = np.asarray(w, dtype=np.float32)
    w_scaled = w.copy()
    w_scaled[PAIRS_OFF:] *= np.float32(0.5)
    w_b = np.ascontiguousarray(np.broadcast_to(w_scaled[None, :], (NI, NF)))
    return [
        {"x": np.ascontiguousarray(x[c * ROWS : (c + 1) * ROWS]), "w": w_b}
        for c in range(NCORES)
    ]


def _run(x, w, trace=False, tmpdir=None):
    from concourse.bass_utils import run_bass_kernel_spmd

    nc = _get_nc()
    in_maps = _prep_in_maps(x, w)
    res = run_bass_kernel_spmd(
        nc, in_maps, list(range(NCORES)), trace=trace, tmpdir=tmpdir
    )
    out = np.concatenate([res.results[c]["out"] for c in range(NCORES)], axis=0)
    return out, res


def kernel(**inputs):
    out, _ = _run(inputs["x"], inputs["w"])
    return out


if __name__ == "__main__":
    for p in PLAN:
        a, gld, dld = p["loads"]
        na = len(p["act"])
        ng = sum(1 for v in p["assign"].values() if v == "G")
        nd = sum(1 for v in p["assign"].values() if v == "D")
        print(
            f"chunk {p['ch']}: A/G/D blocks {na}/{ng}/{nd} wd={p['wd']} "
            f"loads A={a/1e3:6.1f} G={gld/1e3:6.1f} D={dld/1e3:6.1f} us"
        )
    tot = [sum(p["loads"][j] for p in PLAN) / 1e3 for j in range(3)]
    print(f"totals A={tot[0]:.0f} G={tot[1]:.0f} D={tot[2]:.0f} us")


# revision 16
# speedup vs baseline: 1.2387x; 1.0074x over previous
"""CrossProductLayer kernel for Trainium2 (Bass/Tile), 8-core data parallel.

out[b, :] = concat(x[b]**2, x[b], 0.5 * x[b,i]*x[b,j] for i<j) * w

Full inputs:  x [16384, 128] f32, w [8384] f32 -> output [16384, 8384] f32.
Pure batch data parallelism: each of 8 cores computes 2048 rows; w is
pre-scaled (0.5 on the pair block) and pre-broadcast to [128, 8384] host-side.

Per-core kernel: 16 row-tiles of 128 rows. Units = (2 groups of G=8
row-tiles) x (4 column chunks of 2096). Unit tile [128, 8*1048] f32
(33.5 KB/partition, bufs=4). Stores write 4.2 KB HBM rows (~303 GB/s).

Each pair block i (out[:, blk] = x[:,i]*x[:,i+1:]) is one grouped op per
unit over [128, 8, w]. Engine split via measured per-op costs:
  ScalarE: widest blocks of every chunk (per-tile ops), capped budget;
           squares; singles by DMA.
  GpSimdE: narrow blocks (low fixed cost/op) + share of the *w pass.
  VectorE: wide/middle blocks + rest of the *w pass + ScalarE-cols *w.
All VectorE ops are tensor_tensor (1-port) so GpSimdE never contends on
the shared SBUF port; the GpSimd broadcast operand is src1 (src0 step-0
innermost is a slow path). Per-unit *w passes run on the owning engine's
own queue; stores issue early (next loop iteration head).
"""

import numpy as np

B = 16384
NI = 128
NF = NI + NI + (NI * (NI - 1)) // 2  # 8384
NCORES = 8
ROWS = B // NCORES
TILE_P = 128
TILES = ROWS // TILE_P  # 16
PAIRS_OFF = 2 * NI

G = 8
NG = TILES // G  # 2 groups
NCH = 8
CHW = NF // NCH  # 1048

WIDTHS = [NI - 1 - i for i in range(NI - 1)]
STARTS = []
_off = PAIRS_OFF
for _w in WIDTHS:
    STARTS.append(_off)
    _off += _w
assert _off == NF

# fitted per-op costs (ns); per unit (G rows of one group)
ACT_OP = lambda w: 371.0 + 0.83 * w  # per tile => G per unit per block
DVE_OP = lambda w: 620.0 + 1.09 * G * w
GP_OP = lambda w: 390.0 + 1.93 * G * w
DVE_WCOL = 1.09 * G  # per col per unit
GP_WCOL = 1.93 * G

ACT_BUDGET = 160e3  # ns total across all units


def _pieces(ch):
    lo, hi = ch * CHW, (ch + 1) * CHW
    out = []
    for i in range(NI - 1):
        s, w = STARTS[i], WIDTHS[i]
        a, b = max(s, lo), min(s + w, hi)
        if a < b:
            out.append((i, a, b - a))
    return out


def _plan():
    """Per chunk: ACT takes widest (within global cap), GP narrow, DVE
    wide; per-chunk W split balances DVE vs GP."""
    plan = []
    act_per_chunk = ACT_BUDGET / NCH
    for ch in range(NCH):
        ps = _pieces(ch)
        by_w = sorted(ps, key=lambda t: -t[2])
        a_load = 16 * (371 + 128) / 1.2 / NCH if ch == 0 else 0.0
        act_set = set()
        for i, cs, w in by_w:
            c = 16 * ACT_OP(w)
            if a_load + c <= act_per_chunk:
                act_set.add((i, cs))
                a_load += c
        # remaining: GP narrow, DVE wide — threshold by per-op balance
        rest = [(i, cs, w) for i, cs, w in ps if (i, cs) not in act_set]
        d_load, g_load = 0.0, 0.0
        assign = {}
        for i, cs, w in sorted(rest, key=lambda t: -t[2]):
            cd, cg = NG * TILES / G * 0 + 2 * DVE_OP(w), 2 * GP_OP(w)
            if d_load + cd <= g_load + cg:
                assign[(i, cs)] = "D"
                d_load += cd
            else:
                assign[(i, cs)] = "G"
                g_load += cg
        # per-chunk W cols (incl head in chunk 0) split D/G
        wcols = CHW
        wd = (g_load - d_load + 2 * GP_WCOL * wcols) / (
            2 * (DVE_WCOL + GP_WCOL)
        )
        wd = int(np.clip(round(wd), 0, wcols))
        d_load += 2 * DVE_WCOL * wd
        g_load += 2 * GP_WCOL * (wcols - wd)
        plan.append(
            {
                "ch": ch,
                "pieces": ps,
                "act": act_set,
                "assign": assign,
                "wd": wd,
                "loads": (a_load, g_load, d_load),
            }
        )
    return plan


PLAN = _plan()

_CACHE = {}


def _build_nc():
    import os

    os.environ["TILE_EXHAUSTIVE_MEMORY_SHARE_CHECK"] = "1"
    from concourse import bacc
    import concourse.mybir as mybir
    from concourse.tile import TileContext

    f32 = mybir.dt.float32
    nc = bacc.Bacc(
        "TRN2", target_bir_lowering=False, debug=False, num_devices=NCORES
    )
    x_d = nc.dram_tensor("x", [ROWS, NI], f32, kind="ExternalInput")
    w_d = nc.dram_tensor("w", [NI, NF], f32, kind="ExternalInput")
    o_d = nc.dram_tensor("out", [ROWS, NF], f32, kind="ExternalOutput")

    x_hbm3 = x_d.rearrange("(t p) c -> p t c", t=TILES)
    o_hbm3 = o_d.rearrange("(t p) c -> p t c", t=TILES)

    units = [(g, ch) for g in range(NG) for ch in range(NCH)]

    with TileContext(nc) as tc:
        with (
            tc.tile_pool(name="xp", bufs=1) as xp,
            tc.tile_pool(name="wp", bufs=1) as wp,
            tc.tile_pool(name="pp", bufs=4) as pp,
        ):
            x_all = xp.tile([TILE_P, TILES * NI], f32)
            x3 = x_all[:].rearrange("p (t c) -> p t c", t=TILES)
            nc.sync.dma_start(out=x3, in_=x_hbm3)
            w_t = wp.tile([NI, NF], f32)
            nc.sync.dma_start(out=w_t[:], in_=w_d[:])

            pending = []  # (pan3, t0, t1, lo)

            def flush():
                while pending:
                    pan3, t0, t1, lo = pending.pop(0)
                    nc.sync.dma_start(
                        out=o_hbm3[:, t0:t1, lo : lo + CHW], in_=pan3
                    )

            for k, (g, ch) in enumerate(units):
                flush()
                info = PLAN[ch]
                t0, t1 = g * G, (g + 1) * G
                lo = ch * CHW
                pan = pp.tile(
                    [TILE_P, G * CHW], f32, name=f"pan{g}{ch}", tag="pan"
                )
                pan3 = pan[:].rearrange("p (t c) -> p t c", t=G)
                if ch == 0:
                    nc.sync.dma_start(
                        out=pan3[:, :, NI : 2 * NI], in_=x_hbm3[:, t0:t1]
                    )
                    nc.scalar.square(pan3[:, :, 0:NI], x3[:, t0:t1])
                # issue ACT first (longest serial stream), then GP, DVE
                for eng_sel in ("A", "G", "D"):
                    for i, cs, w in info["pieces"]:
                        key = (i, cs)
                        e = (
                            "A"
                            if key in info["act"]
                            else info["assign"][key]
                        )
                        if e != eng_sel:
                            continue
                        c = cs - lo
                        j0 = i + 1 + (cs - STARTS[i])
                        src = x3[:, t0:t1, j0 : j0 + w]
                        dst = pan3[:, :, c : c + w]
                        if e == "A":
                            for t in range(G):
                                nc.scalar.mul(
                                    dst[:, t],
                                    src[:, t],
                                    x3[:, t0 + t, i : i + 1],
                                )
                        else:
                            bc = x3[:, t0:t1, i : i + 1].broadcast_to(
                                [TILE_P, G, w]
                            )
                            if e == "G":
                                nc.gpsimd.tensor_mul(dst, src, bc)
                            else:
                                nc.vector.tensor_mul(dst, bc, src)
                # *w pass split D/G by the planner
                wd = info["wd"]
                wsl = w_t[:, None, lo : lo + CHW]
                if wd > 0:
                    nc.vector.tensor_mul(
                        pan3[:, :, 0:wd],
                        pan3[:, :, 0:wd],
                        wsl[:, :, 0:wd].broadcast_to([TILE_P, G, wd]),
                    )
                if wd < CHW:
                    nc.gpsimd.tensor_mul(
                        pan3[:, :, wd:CHW],
                        pan3[:, :, wd:CHW],
                        wsl[:, :, wd:CHW].broadcast_to(
                            [TILE_P, G, CHW - wd]
                        ),
                    )
                pending.append((pan3, t0, t1, lo))
            flush()
    nc.compile()
    return nc


def _get_nc():
    if "nc" not in _CACHE:
        _CACHE["nc"] = _build_nc()
    return _CACHE["nc"]


def _prep_in_maps(x, w):
    x = np.ascontiguousarray(np.asarray(x, dtype=np.float32))
    w = np.asarray(w, dtype=np.float32)
    w_scaled = w.copy()
    w_scaled[PAIRS_OFF:] *= np.float32(0.5)
    w_b = np.ascontiguousarray(np.broadcast_to(w_scaled[None, :], (NI, NF)))
    return [
        {"x": np.ascontiguousarray(x[c * ROWS : (c + 1) * ROWS]), "w": w_b}
        for c in range(NCORES)
    ]


def _run(x, w, trace=False, tmpdir=None):
    from concourse.bass_utils import run_bass_kernel_spmd

    nc = _get_nc()
    in_maps = _prep_in_maps(x, w)
    res = run_bass_kernel_spmd(
        nc, in_maps, list(range(NCORES)), trace=trace, tmpdir=tmpdir
    )
    out = np.concatenate([res.results[c]["out"] for c in range(NCORES)], axis=0)
    return out, res


def kernel(**inputs):
    out, _ = _run(inputs["x"], inputs["w"])
    return out


if __name__ == "__main__":
    for p in PLAN:
        a, gld, dld = p["loads"]
        na = len(p["act"])
        ng = sum(1 for v in p["assign"].values() if v == "G")
        nd = sum(1 for v in p["assign"].values() if v == "D")
        print(
            f"chunk {p['ch']}: A/G/D blocks {na}/{ng}/{nd} wd={p['wd']} "
            f"loads A={a/1e3:6.1f} G={gld/1e3:6.1f} D={dld/1e3:6.1f} us"
        )
    tot = [sum(p["loads"][j] for p in PLAN) / 1e3 for j in range(3)]
    print(f"totals A={tot[0]:.0f} G={tot[1]:.0f} D={tot[2]:.0f} us")


# revision 17
# speedup vs baseline: 1.2942x; 1.0448x over previous
"""CrossProductLayer kernel for Trainium2 (Bass/Tile), 8-core data parallel.

out[b, :] = concat(x[b]**2, x[b], 0.5 * x[b,i]*x[b,j] for i<j) * w

Full inputs:  x [16384, 128] f32, w [8384] f32.
Full output:  [16384, 8384] f32.

Sharding: pure data parallel on the batch dim — each of the 8 cores gets
2048 rows of x; w (pre-scaled and pre-broadcast to [128, 8384] on host) is
replicated. No collectives needed (forward only).

Per-core device kernel (16 row-tiles of 128 batch rows):
  - squares  -> ScalarE (Square activation)
  - singles  -> DMA'd straight from HBM into the output tile
  - pairs    -> per-i blocks out[:, blk_i] = x[:, i] * x[:, i+1:]:
               wide blocks (i < K_ACT) on ScalarE via activation scale,
               the rest on VectorE tensor_scalar (fp32 2x mode; odd widths
               padded by one column which the next block overwrites)
  - *w pass  -> one full-width VectorE tensor_tensor multiply
  - store    -> one 4.3 MB HWDGE DMA per tile
"""

import numpy as np

B = 16384
NI = 128
NF = NI + NI + (NI * (NI - 1)) // 2  # 8384
NCORES = 8
ROWS = B // NCORES  # 2048
TILE_P = 128
TILES = ROWS // TILE_P  # 16
PAIRS_OFF = 2 * NI  # 256
K_ACT = 52  # pair blocks 0..K_ACT-1 run on ScalarE, the rest on VectorE

_CACHE = {}


def _build_nc():
    from concourse import bacc
    import concourse.mybir as mybir
    from concourse.tile import TileContext

    f32 = mybir.dt.float32
    nc = bacc.Bacc(
        "TRN2",
        target_bir_lowering=False,
        debug=False,
        num_devices=NCORES,
    )
    x_d = nc.dram_tensor("x", [ROWS, NI], f32, kind="ExternalInput")
    w_d = nc.dram_tensor("w", [NI, NF], f32, kind="ExternalInput")
    o_d = nc.dram_tensor("out", [ROWS, NF], f32, kind="ExternalOutput")

    with TileContext(nc) as tc:
        with (
            tc.tile_pool(name="wp", bufs=1) as wp,
            tc.tile_pool(name="xp", bufs=4) as xp,
            tc.tile_pool(name="op", bufs=3) as op,
        ):
            w_t = wp.tile([NI, NF], f32)
            nc.sync.dma_start(out=w_t[:], in_=w_d[:])
            for t in range(TILES):
                r0 = t * TILE_P
                x_t = xp.tile([TILE_P, NI + 2], f32)
                nc.sync.dma_start(out=x_t[:, 0:NI], in_=x_d[r0 : r0 + TILE_P])
                # output tile; 16 spare cols so the last padded pair block
                # can spill one column past NF
                o_t = op.tile([TILE_P, NF + 16], f32)
                # singles block [NI:2*NI) comes straight from HBM
                nc.sync.dma_start(out=o_t[:, NI : 2 * NI], in_=x_d[r0 : r0 + TILE_P])
                # squares block [0:NI)
                nc.scalar.square(o_t[:, 0:NI], x_t[:, 0:NI])
                off = PAIRS_OFF
                for i in range(NI - 1):
                    wdt = NI - 1 - i
                    sc = x_t[:, i : i + 1]
                    if i < K_ACT:
                        nc.scalar.mul(
                            o_t[:, off : off + wdt], x_t[:, i + 1 : i + 1 + wdt], sc
                        )
                    else:
                        # pad odd widths to even for the DVE fp32 2x mode;
                        # the padded column is overwritten by block i+1
                        wpad = wdt + (wdt & 1)
                        nc.vector.tensor_scalar_mul(
                            o_t[:, off : off + wpad],
                            x_t[:, i + 1 : i + 1 + wpad],
                            sc,
                        )
                    off += wdt
                # the *w pass over the whole tile
                nc.vector.tensor_mul(o_t[:, 0:NF], o_t[:, 0:NF], w_t[:])
                nc.sync.dma_start(out=o_d[r0 : r0 + TILE_P], in_=o_t[:, 0:NF])
    nc.compile()
    return nc


def _get_nc():
    if "nc" not in _CACHE:
        _CACHE["nc"] = _build_nc()
    return _CACHE["nc"]


def _prep_in_maps(x, w):
    x = np.ascontiguousarray(np.asarray(x, dtype=np.float32))
    w = np.asarray(w, dtype=np.float32)
    w_scaled = w.copy()
    w_scaled[PAIRS_OFF:] *= np.float32(0.5)
    w_b = np.ascontiguousarray(np.broadcast_to(w_scaled[None, :], (NI, NF)))
    return [
        {"x": np.ascontiguousarray(x[c * ROWS : (c + 1) * ROWS]), "w": w_b}
        for c in range(NCORES)
    ]


def _run(x, w, trace=False, tmpdir=None):
    from concourse.bass_utils import run_bass_kernel_spmd

    nc = _get_nc()
    in_maps = _prep_in_maps(x, w)
    res = run_bass_kernel_spmd(
        nc, in_maps, list(range(NCORES)), trace=trace, tmpdir=tmpdir
    )
    out = np.concatenate([res.results[c]["out"] for c in range(NCORES)], axis=0)
    return out, res


def kernel(**inputs):
    out, _ = _run(inputs["x"], inputs["w"])
    return out


# revision 20
# speedup vs baseline: 1.2963x; 1.0016x over previous
"""CrossProductLayer kernel for Trainium2 (Bass/Tile), 8-core data parallel.

out[b, :] = concat(x[b]**2, x[b], 0.5 * x[b,i]*x[b,j] for i<j) * w

Full inputs:  x [16384, 128] f32, w [8384] f32.
Full output:  [16384, 8384] f32.

Sharding: pure data parallel on the batch dim — each of the 8 cores gets
2048 rows of x; w (pre-scaled and pre-broadcast to [128, 8384] on host) is
replicated. No collectives needed (forward only).

Per-core device kernel (16 row-tiles of 128 batch rows):
  - squares  -> ScalarE (Square activation)
  - singles  -> DMA'd straight from HBM into the output tile
  - pairs    -> per-i blocks out[:, blk_i] = x[:, i] * x[:, i+1:]:
               wide blocks (i < K_ACT) on ScalarE via activation scale,
               the rest on VectorE tensor_scalar (fp32 2x mode; odd widths
               padded by one column which the next block overwrites)
  - *w pass  -> one full-width VectorE tensor_tensor multiply
  - store    -> one 4.3 MB HWDGE DMA per tile
"""

import numpy as np

B = 16384
NI = 128
NF = NI + NI + (NI * (NI - 1)) // 2  # 8384
NCORES = 8
ROWS = B // NCORES  # 2048
TILE_P = 128
TILES = ROWS // TILE_P  # 16
PAIRS_OFF = 2 * NI  # 256
K_ACT = 53  # pair blocks 0..K_ACT-1 run on ScalarE, the rest on VectorE

_CACHE = {}


def _build_nc():
    import os

    # precise (unbounded) overlap tracking: the padded TS blocks and the
    # half-tile *w passes need byte-range-accurate deps, not the
    # conservative fallback past 100 pairwise checks
    os.environ["TILE_EXHAUSTIVE_MEMORY_SHARE_CHECK"] = "1"
    from concourse import bacc
    import concourse.mybir as mybir
    from concourse.tile import TileContext

    f32 = mybir.dt.float32
    nc = bacc.Bacc(
        "TRN2",
        target_bir_lowering=False,
        debug=False,
        num_devices=NCORES,
    )
    x_d = nc.dram_tensor("x", [ROWS, NI], f32, kind="ExternalInput")
    w_d = nc.dram_tensor("w", [NI, NF], f32, kind="ExternalInput")
    o_d = nc.dram_tensor("out", [ROWS, NF], f32, kind="ExternalOutput")

    with TileContext(nc) as tc:
        with (
            tc.tile_pool(name="wp", bufs=1) as wp,
            tc.tile_pool(name="xp", bufs=4) as xp,
            tc.tile_pool(name="op", bufs=4) as op,
        ):
            w_t = wp.tile([NI, NF], f32)
            nc.sync.dma_start(out=w_t[:], in_=w_d[:])
            for t in range(TILES):
                r0 = t * TILE_P
                x_t = xp.tile([TILE_P, NI + 2], f32)
                nc.sync.dma_start(out=x_t[:, 0:NI], in_=x_d[r0 : r0 + TILE_P])
                # output tile; 16 spare cols so the last padded pair block
                # can spill one column past NF
                o_t = op.tile([TILE_P, NF + 16], f32)
                # singles block [NI:2*NI) comes straight from HBM
                nc.sync.dma_start(out=o_t[:, NI : 2 * NI], in_=x_d[r0 : r0 + TILE_P])
                # squares block [0:NI)
                nc.scalar.square(o_t[:, 0:NI], x_t[:, 0:NI])
                off = PAIRS_OFF
                for i in range(NI - 1):
                    wdt = NI - 1 - i
                    sc = x_t[:, i : i + 1]
                    if i < K_ACT:
                        nc.scalar.mul(
                            o_t[:, off : off + wdt], x_t[:, i + 1 : i + 1 + wdt], sc
                        )
                    else:
                        # pad odd widths to even for the DVE fp32 2x mode;
                        # the padded column is overwritten by block i+1
                        wpad = wdt + (wdt & 1)
                        nc.vector.tensor_scalar_mul(
                            o_t[:, off : off + wpad],
                            x_t[:, i + 1 : i + 1 + wpad],
                            sc,
                        )
                    off += wdt
                # the *w pass and store in two halves: the first half's
                # store can start while the second half is still being
                # multiplied (16.8 KB HBM rows stay at full DMA rate)
                H = NF // 2
                nc.vector.tensor_mul(o_t[:, 0:H], o_t[:, 0:H], w_t[:, 0:H])
                nc.sync.dma_start(
                    out=o_d[r0 : r0 + TILE_P, 0:H], in_=o_t[:, 0:H]
                )
                nc.vector.tensor_mul(o_t[:, H:NF], o_t[:, H:NF], w_t[:, H:NF])
                nc.sync.dma_start(
                    out=o_d[r0 : r0 + TILE_P, H:NF], in_=o_t[:, H:NF]
                )
    nc.compile()
    return nc


def _get_nc():
    if "nc" not in _CACHE:
        _CACHE["nc"] = _build_nc()
    return _CACHE["nc"]


def _prep_in_maps(x, w):
    x = np.ascontiguousarray(np.asarray(x, dtype=np.float32))
    w = np.asarray(w, dtype=np.float32)
    w_scaled = w.copy()
    w_scaled[PAIRS_OFF:] *= np.float32(0.5)
    w_b = np.ascontiguousarray(np.broadcast_to(w_scaled[None, :], (NI, NF)))
    return [
        {"x": np.ascontiguousarray(x[c * ROWS : (c + 1) * ROWS]), "w": w_b}
        for c in range(NCORES)
    ]


def _run(x, w, trace=False, tmpdir=None):
    from concourse.bass_utils import run_bass_kernel_spmd

    nc = _get_nc()
    in_maps = _prep_in_maps(x, w)
    res = run_bass_kernel_spmd(
        nc, in_maps, list(range(NCORES)), trace=trace, tmpdir=tmpdir
    )
    out = np.concatenate([res.results[c]["out"] for c in range(NCORES)], axis=0)
    return out, res


def kernel(**inputs):
    out, _ = _run(inputs["x"], inputs["w"])
    return out
